# revision 13
# baseline (speedup 1.0000x reference)
"""Bass/Tile kernel for the sharded NT-Xent contrastive loss (v3, fp8).

Per-core computation (core c of 8), B=8192 D=512 M=1024:
  inputs (host pre-packed, bf16, d-pair-interleaved for DoubleRow):
    vt [128, 4, 1024] bf16 : vt[p, dt, i]  = v[c*M+i, dt*128+p]
    ut [128, 4, 8192] bf16 : ut[p, dt, j]  = u[j,     dt*128+p]
    us [128, 4, 1024] bf16 : us[p, dt, i]  = u[c*M+i, dt*128+p]
  output:
    loss [1024] f32 : loss rows c*M:(c+1)*M

  math (alpha=16 fp8 scaling folded into the exp/ln rsqrt path):
    inv16u[j] = 16/||u_j||      (rsqrt via exp(-0.5*ln(ss)+ln 16))
    ub8 = ut * inv16u  (fp8)    vb8 = 16 * vt  (fp8)
    S[i,j]  = sum_d vb8[d,i]*ub8[d,j]          (PE fp8 DoubleRow, psum f32)
            = 256 * (v_i . u_j/||u_j||)
    sv[i]   = exp(-0.5*ln(ssv) + ln(2/256))  = 2/(256*||v_i||)
    den[i]  = sum_j exp(S[i,j]*sv[i])          (ACT exp, accum_out)
    dot[i]  = sum_d vt[d,i]*us[d,i]  (bf16 products, f32 psum colsum)
    s2[i]   = exp(-0.5*(ln ssv + ln ssus) + ln 2) = 2/(||v_i|| ||u_i||)
    t2[i]   = s2[i]*dot[i] = 2*cos(v_i, u_i)
    loss[i] = log(exp(t2) + den) - t2

The u-chunk pipeline (4 chunks of 2048 cols) overlaps u DMA + prep
(square/colsum/rsqrt/normalize) with the main matmul+exp loop on
earlier chunks.  Only Exp/Ln activations are used -> one table set.
"""

import math
from contextlib import ExitStack

import concourse.bass as bass
import concourse.tile as tile
from concourse import bacc, mybir

F32 = mybir.dt.float32
BF16 = mybir.dt.bfloat16
FP8 = mybir.dt.float8e4
MULT = mybir.AluOpType.mult
ADD = mybir.AluOpType.add
SUB = mybir.AluOpType.subtract
AF = mybir.ActivationFunctionType
DR = mybir.MatmulPerfMode.DoubleRow

B = 8192
D = 512
NCORES = 8
M = B // NCORES          # 1024
KT = D // 128            # 4 d-tiles
NIT = M // 128           # 8 i-tiles
CHUNK = 2048             # u-cols per pipeline chunk
NCH = B // CHUNK         # 4 chunks
NSUB = CHUNK // 512      # 4 colsum subchunks per chunk
ALPHA = 16.0             # fp8 scale on both operands
LN_A = math.log(ALPHA)


def build_nc():
    nc = bacc.Bacc("TRN2", target_bir_lowering=False, debug=False,
                   num_devices=NCORES)

    vt = nc.dram_tensor("vt", [128, KT, M], BF16, kind="ExternalInput")
    ut = nc.dram_tensor("ut", [128, KT, B], BF16, kind="ExternalInput")
    us = nc.dram_tensor("us", [128, KT, M], BF16, kind="ExternalInput")
    loss = nc.dram_tensor("loss", [M], F32, kind="ExternalOutput")
    # DRAM bounce buffers (v-prep compacts only)
    bv_d = [nc.dram_tensor(f"bv{k}", [M], F32) for k in range(3)]

    with tile.TileContext(nc) as tc, ExitStack() as ctx:
        consts = ctx.enter_context(tc.tile_pool(name="consts", bufs=1))
        upool = ctx.enter_context(tc.tile_pool(name="upool", bufs=1))
        vpool = ctx.enter_context(tc.tile_pool(name="vpool", bufs=1))
        keep = ctx.enter_context(tc.tile_pool(name="keep", bufs=1))
        stage = ctx.enter_context(tc.tile_pool(name="stage", bufs=2))
        sqp = ctx.enter_context(tc.tile_pool(name="sqp", bufs=2))
        invp = ctx.enter_context(tc.tile_pool(name="invp", bufs=2))
        expp = ctx.enter_context(tc.tile_pool(name="expp", bufs=2))
        smallp = ctx.enter_context(tc.tile_pool(name="smallp", bufs=2))

        ones_bf = consts.tile([128, 128], BF16)
        nc.vector.memset(ones_bf[:], 1.0)
        # bias constants for the exp/ln rsqrt paths
        b_sv = consts.tile([128, 1], F32, name="b_sv")
        nc.vector.memset(b_sv[:], math.log(2.0) - 2.0 * LN_A)
        b_ln2 = consts.tile([128, 1], F32, name="b_ln2")
        nc.vector.memset(b_ln2[:], math.log(2.0))
        b_lna = consts.tile([128, 1], F32, name="b_lna")
        nc.vector.memset(b_lna[:], LN_A)
        # preload the combined exp+ln table set once; the auto-insertion
        # pass then sees every activation covered (no per-chunk thrash)
        nc.scalar.add_instruction(mybir.InstLoadActFuncSet(
            name=nc.get_next_instruction_name(), ins=[], outs=[],
            act_func_set_id=6))

        ub8 = upool.tile([128, KT, B], FP8, name="ub8")
        vb8 = vpool.tile([128, KT, M], FP8, name="vb8")

        sv = keep.tile([128, NIT], F32)       # exp scale 2/(256 ||v_i||)
        t2 = keep.tile([128, NIT], F32)       # 2 cos(v_i, u_i)
        dp_all = keep.tile([128, NIT * NCH], F32)  # accum_out slots

        # ================= v-prep (prologue) =================
        with tc.tile_pool(name="vstage", bufs=1) as vst, \
             tc.tile_pool(name="vps", bufs=1, space="PSUM") as vps:
            vsg = vst.tile([128, KT, M], BF16, tag="vsg")
            usg = vst.tile([128, KT, M], BF16, tag="usg")
            nc.sync.dma_start(vsg[:], vt.ap())
            nc.sync.dma_start(usg[:], us.ap())

            ps_v = vps.tile([128, M], F32, tag="psv")
            ps_us = vps.tile([128, M], F32, tag="psus")
            ps_dot = vps.tile([128, M], F32, tag="psdot")
            for dt in range(KT):
                # cast v to fp8 with alpha scale
                nc.vector.tensor_scalar(vb8[:, dt, :], vsg[:, dt, :],
                                        ALPHA, None, MULT)
                for name, ps_acc, a, b_ in (
                    ("v2", ps_v, vsg, vsg),
                    ("u2", ps_us, usg, usg),
                    ("vu", ps_dot, vsg, usg),
                ):
                    pr = vst.tile([128, M], BF16, tag=f"pr{name}",
                                  name=f"pr{name}{dt}", bufs=2)
                    nc.vector.tensor_tensor(pr[:], a[:, dt, :], b_[:, dt, :],
                                            MULT)
                    for jc in range(M // 512):
                        nc.tensor.matmul(
                            ps_acc[:, jc * 512:(jc + 1) * 512],
                            lhsT=ones_bf[:],
                            rhs=pr[:, jc * 512:(jc + 1) * 512],
                            start=(dt == 0), stop=(dt == KT - 1))
            # bounce [1, M] rows -> compact [128, NIT]
            ssv_c = smallp.tile([128, NIT], F32, tag="ssv", bufs=1)
            ssus_c = smallp.tile([128, NIT], F32, tag="ssus", bufs=1)
            dot_c = smallp.tile([128, NIT], F32, tag="dot", bufs=1)
            for k, (ps_acc, dst) in enumerate(
                    ((ps_v, ssv_c), (ps_us, ssus_c), (ps_dot, dot_c))):
                fl = vst.tile([1, M], F32, tag="fl", name=f"fl{k}", bufs=3)
                nc.vector.tensor_copy(fl[:], ps_acc[0:1, :])
                nc.sync.dma_start(bv_d[k].ap(), fl[:])
                nc.sync.dma_start(
                    dst[:], bv_d[k].ap().rearrange("(t p) -> p t", p=128))

            # sv = exp(-0.5 ln ssv + ln(2/alpha^2)); s2 = exp(-.5(lv+lus)+ln2)
            lv = smallp.tile([128, NIT], F32, tag="lv", bufs=1)
            lus = smallp.tile([128, NIT], F32, tag="lus", bufs=1)
            nc.scalar.activation(lv[:], ssv_c[:], AF.Ln)
            nc.scalar.activation(lus[:], ssus_c[:], AF.Ln)
            nc.scalar.activation(sv[:], lv[:], AF.Exp, scale=-0.5,
                                 bias=b_sv[:])
            lsum = smallp.tile([128, NIT], F32, tag="lsum", bufs=1)
            nc.vector.tensor_tensor(lsum[:], lv[:], lus[:], ADD)
            s2 = smallp.tile([128, NIT], F32, tag="s2", bufs=1)
            nc.scalar.activation(s2[:], lsum[:], AF.Exp, scale=-0.5,
                                 bias=b_ln2[:])
            nc.vector.tensor_tensor(t2[:], dot_c[:], s2[:], MULT)

        # ================= main loop with u-chunk pipeline =================
        def stage_chunk(ch):
            """DMA u chunk ch into a stage tile and square it."""
            sg = stage.tile([128, KT, CHUNK], BF16, tag="usg",
                            name=f"usg{ch}")
            sq = sqp.tile([128, KT, CHUNK], BF16, tag="usq",
                          name=f"usq{ch}")
            for dt in range(KT):
                nc.sync.dma_start(
                    sg[:, dt, :], ut.ap()[:, dt, ch * CHUNK:(ch + 1) * CHUNK])
                nc.vector.tensor_tensor(sq[:, dt, :], sg[:, dt, :],
                                        sg[:, dt, :], MULT)
            return sg, sq

        def prep_colsum(ps, sq, ch):
            """Column sum-of-squares for chunk ch into main psum tile ps."""
            for s in range(NSUB):
                sl = slice(s * 512, (s + 1) * 512)
                for dt in range(KT):
                    nc.tensor.matmul(ps[:, sl], lhsT=ones_bf[:],
                                     rhs=sq[:, dt, sl],
                                     start=(dt == 0), stop=(dt == KT - 1))

        def prep_finish(sg, ps, ch):
            """Full-width rsqrt from psum (replicated), normalize into ub8."""
            ls = invp.tile([128, CHUNK], F32, tag="ls", name=f"ls{ch}")
            nc.scalar.activation(ls[:], ps[:], AF.Ln)
            ivr = invp.tile([128, CHUNK], BF16, tag="ivr", name=f"ivr{ch}")
            nc.scalar.activation(ivr[:], ls[:], AF.Exp, scale=-0.5,
                                 bias=b_lna[:])
            for dt in range(KT):
                nc.vector.tensor_tensor(
                    ub8[:, dt, ch * CHUNK:(ch + 1) * CHUNK],
                    sg[:, dt, :], ivr[:], MULT)

        with tc.tile_pool(name="mpsum", bufs=2, space="PSUM") as mps:
            # chunk 0 prep (uses a main-pool psum tile before round 0)
            sg0, sq0 = stage_chunk(0)
            ps = mps.tile([128, CHUNK], F32, tag="mm")
            prep_colsum(ps, sq0, 0)
            prep_finish(sg0, ps, 0)
            sg_next, sq_next = stage_chunk(1)

            for ch in range(NCH):
                for it in range(NIT):
                    # mid-round, prep next chunk in a dedicated acquisition
                    if it == 6 and ch + 1 < NCH:
                        psp = mps.tile([128, CHUNK], F32, tag="mm")
                        prep_colsum(psp, sq_next, ch + 1)
                        prep_finish(sg_next, psp, ch + 1)
                        if ch + 2 < NCH:
                            sg_next, sq_next = stage_chunk(ch + 2)
                    ps = mps.tile([128, CHUNK], F32, tag="mm")
                    for kp in range(KT // 2):
                        for jj in range(NSUB):
                            j0 = ch * CHUNK + jj * 512
                            nc.tensor.matmul(
                                ps[:, jj * 512:(jj + 1) * 512],
                                lhsT=vb8[:, 2 * kp:2 * kp + 2,
                                         it * 128:(it + 1) * 128],
                                rhs=ub8[:, 2 * kp:2 * kp + 2, j0:j0 + 512],
                                start=(kp == 0), stop=(kp == KT // 2 - 1),
                                perf_mode=DR)
                    ex = expp.tile([128, CHUNK], BF16, tag="ex")
                    nc.scalar.activation(
                        ex[:], ps[:], AF.Exp,
                        scale=sv[:, it:it + 1],
                        accum_out=dp_all[:, it * NCH + ch:it * NCH + ch + 1])

            # ================= epilogue =================
            den = smallp.tile([128, NIT], F32, tag="den", bufs=1)
            dp_v = dp_all[:].rearrange("p (t c) -> p t c", c=NCH)
            nc.vector.tensor_tensor(den[:], dp_v[:, :, 0], dp_v[:, :, 1], ADD)
            for r in range(2, NCH):
                nc.vector.tensor_tensor(den[:], den[:], dp_v[:, :, r], ADD)
            numt = smallp.tile([128, NIT], F32, tag="numt", bufs=1)
            nc.scalar.activation(numt[:], t2[:], AF.Exp)
            dtot = smallp.tile([128, NIT], F32, tag="dtot", bufs=1)
            nc.vector.tensor_tensor(dtot[:], den[:], numt[:], ADD)
            lg = smallp.tile([128, NIT], F32, tag="lg", bufs=1)
            nc.scalar.activation(lg[:], dtot[:], AF.Ln)
            lt = smallp.tile([128, NIT], F32, tag="lt", bufs=1)
            nc.vector.tensor_tensor(lt[:], lg[:], t2[:], SUB)
            nc.sync.dma_start(
                loss.ap().rearrange("(t p) -> p t", p=128), lt[:])

    nc.compile()
    return nc


# ======================================================================
# Host-side entry point: full inputs in, full output out.
# ======================================================================
import numpy as np

_NC_CACHE = {}


def _get_nc():
    if "nc" not in _NC_CACHE:
        _NC_CACHE["nc"] = build_nc()
    return _NC_CACHE["nc"]


def _pack(xT: np.ndarray) -> np.ndarray:
    """[D, N] f32 -> [128, KT, N] bf16 with xP[p, dt, n] = xT[dt*128+p, n]."""
    import ml_dtypes
    xP = xT.reshape(KT, 128, xT.shape[1]).transpose(1, 0, 2)
    return np.ascontiguousarray(xP.astype(ml_dtypes.bfloat16))


def _make_in_maps(v: np.ndarray, u: np.ndarray) -> list:
    v = np.asarray(v, dtype=np.float32)
    u = np.asarray(u, dtype=np.float32)
    vT = np.ascontiguousarray(v.T)          # [D, B]
    uT = np.ascontiguousarray(u.T)          # [D, B]
    utp = _pack(uT)
    in_maps = []
    for c in range(NCORES):
        sl = slice(c * M, (c + 1) * M)
        in_maps.append({
            "vt": _pack(vT[:, sl]),
            "ut": utp,
            "us": np.ascontiguousarray(utp[:, :, sl]),
        })
    return in_maps


def kernel(v: np.ndarray, u: np.ndarray) -> np.ndarray:
    from concourse.bass_utils import run_bass_kernel_spmd

    nc = _get_nc()
    in_maps = _make_in_maps(v, u)
    res = run_bass_kernel_spmd(nc, in_maps, core_ids=list(range(NCORES)))
    return np.concatenate([res.results[c]["loss"] for c in range(NCORES)])
